# revision 6
# baseline (speedup 1.0000x reference)
"""BalSCL (balanced supervised contrastive loss) for Trainium2, 8 NeuronCores.

Math restructure (avoids all per-element mask work on device):
  tgt  = [targets, targets, arange(C)]            (length J = 2B + C = 8292)
  feats = [view0, view1, centers]                 [J, D], L2-normalized rows
  S[j, i] = feats_j . feats_i  (i over the 2B anchors), l = S / t  (offset-0
  convention: the reference's row-max subtraction cancels identically in
  loss_i = log(sum_j w_ij e^{l_ij}) - (sum_j mask_ij l_ij) / M_i).

  Reference per-row sum:  A_i = sum_{j != i} e^{l_ij} / (cnt[tgt_j] - m_ij)
  with m_ij = [tgt_j == tgt_i].  Since the weight only depends on (class of j,
  whether it equals class of i), A_i is a function of per-class exp-sums:
      E1[k, i] = sum_{j: tgt_j = k} e^{l_ij}
      A_i = sum_k E1[k,i]/cnt[k] + (1/(cnt[t_i]-1) - 1/cnt[t_i]) * E1[t_i, i]
            - e^{l_ii} / (cnt[t_i] - 1)
  The device computes ONLY E1 (matmul -> exp -> one-hot matmul); everything
  else (tiny O(B + C*B) work) happens on host in float64.

  The mask*logits sum is exact on host: sum_j mask_ij S_ij = f_i . G[t_i] - S_ii
  with G = per-class feature sums.

Device inputs are pre-truncated to FP22 (the PE's float32r multiply precision)
so the host knows the matmul inputs bit-exactly and can reproduce the e^{l_ii}
diagonal term it must subtract.

Sharding: anchors (the 2B = 8192 logit rows; free axis i on device) are split
1024 per core; feats^T / one-hot are replicated; host sums the scalar.
"""

import numpy as np

C = 100
B = 4096
D = 128
TWOB = 2 * B
J = TWOB + C            # 8292
NCHUNK = 65
JPAD = NCHUNK * 128     # 8320
NCORES = 8
PER = TWOB // NCORES    # 1024 anchors per core
INVT = 10.0             # 1 / temperature
EII_MODE = "trunc"      # how the device rounded the diagonal exp (calibrated)

_NC_CACHE = {}


def _truncate_fp22(a: np.ndarray) -> np.ndarray:
    """Truncate f32 mantissa to 13 bits (FP22) — what the PE reads for f32r."""
    b = np.ascontiguousarray(a, dtype=np.float32).copy()
    b.view(np.uint32)[...] &= np.uint32(0xFFFFFC00)
    return b


def _build_nc():
    import concourse.bacc as bacc
    import concourse.mybir as mybir
    import concourse.tile as tile

    f32 = mybir.dt.float32
    f32r = mybir.dt.float32r
    Exp = mybir.ActivationFunctionType.Exp

    nc = bacc.Bacc("TRN2", target_bir_lowering=False, debug=False,
                   num_devices=NCORES)

    ft_d = nc.dram_tensor("featsT", [D, JPAD], f32r, kind="ExternalInput")
    # one-hot pre-swizzled on host to SBUF layout [p, c*C + k] = onehot[128c+p, k]
    oh_d = nc.dram_tensor("onehot", [128, NCHUNK * C], f32r, kind="ExternalInput")
    an_d = nc.dram_tensor("anch", [D, PER], f32r, kind="ExternalInput")
    e1_d = nc.dram_tensor("e1", [C, PER], f32, kind="ExternalOutput")

    with tile.TileContext(nc) as tc:
        with (
            tc.tile_pool(name="big", bufs=1) as big,
            tc.tile_pool(name="epool", bufs=3) as epool,
            tc.tile_pool(name="outp", bufs=1) as outp,
            tc.tile_pool(name="spool", bufs=2, space="PSUM") as spool,
            tc.tile_pool(name="accpool", bufs=1, space="PSUM") as accpool,
        ):
            an = big.tile([D, PER], f32r, name="an")
            nc.sync.dma_start(out=an, in_=an_d[:, :])

            ft = big.tile([D, JPAD], f32r, name="ft")
            FT_SPLIT = 4
            fper = JPAD // FT_SPLIT
            for s in range(FT_SPLIT):
                nc.sync.dma_start(out=ft[:, s * fper:(s + 1) * fper],
                                  in_=ft_d[:, s * fper:(s + 1) * fper])

            oh = big.tile([128, NCHUNK * C], f32r, name="oh")
            OH_SPLIT = 8
            bounds = [round(NCHUNK * s / OH_SPLIT) for s in range(OH_SPLIT + 1)]
            for s in range(OH_SPLIT):
                a, b = bounds[s] * C, bounds[s + 1] * C
                nc.scalar.dma_start(out=oh[:, a:b], in_=oh_d[:, a:b])

            E1a = accpool.tile([C, 512], f32, name="E1a")
            E1b = accpool.tile([C, 512], f32, name="E1b")

            def emit_reduce(c, e):
                lhs = oh[:, c * C:(c + 1) * C]
                nc.tensor.matmul(E1a, lhsT=lhs, rhs=e[:, 0:512],
                                 start=(c == 0), stop=(c == NCHUNK - 1),
                                 skip_group_check=True)
                nc.tensor.matmul(E1b, lhsT=lhs, rhs=e[:, 512:1024],
                                 start=(c == 0), stop=(c == NCHUNK - 1),
                                 skip_group_check=True)

            prev = None
            for c in range(NCHUNK):
                S = spool.tile([128, 1024], f32, name="S")
                lhs = ft[:, c * 128:(c + 1) * 128]
                nc.tensor.matmul(S[:, 0:512], lhsT=lhs,
                                 rhs=an[:, 0:512],
                                 start=True, stop=True)
                nc.tensor.matmul(S[:, 512:1024], lhsT=lhs,
                                 rhs=an[:, 512:1024],
                                 start=True, stop=True)
                e = epool.tile([128, 1024], f32r, name="e")
                nc.scalar.activation(out=e, in_=S, func=Exp,
                                     bias=0.0, scale=INVT)
                # software-pipeline skew: reduce for chunk c-1 is emitted after
                # chunk c's main matmuls so the PE never waits on the exp
                if prev is not None:
                    emit_reduce(*prev)
                prev = (c, e)
            emit_reduce(*prev)

            out_sb = outp.tile([C, PER], f32, name="out_sb")
            nc.vector.tensor_copy(out=out_sb[:, 0:512], in_=E1a)
            nc.vector.tensor_copy(out=out_sb[:, 512:1024], in_=E1b)
            nc.sync.dma_start(out=e1_d[:, :], in_=out_sb)

    nc.compile()
    return nc


def get_nc():
    if "nc" not in _NC_CACHE:
        _NC_CACHE["nc"] = _build_nc()
    return _NC_CACHE["nc"]


def _device_e1(featsT_pad: np.ndarray, oh_sw: np.ndarray) -> np.ndarray:
    """Run the SPMD kernel on 8 cores; return E1 [C, 2B] float32."""
    from concourse.bass_utils import run_bass_kernel_spmd

    nc = get_nc()
    in_maps = []
    for core in range(NCORES):
        anch = np.ascontiguousarray(featsT_pad[:, core * PER:(core + 1) * PER])
        in_maps.append({"featsT": featsT_pad, "onehot": oh_sw, "anch": anch})
    res = run_bass_kernel_spmd(nc, in_maps, core_ids=list(range(NCORES)))
    return np.concatenate([res.results[c]["e1"] for c in range(NCORES)], axis=1)


def kernel(centers1: np.ndarray, features: np.ndarray,
           targets: np.ndarray) -> np.ndarray:
    centers1 = np.asarray(centers1, dtype=np.float32)
    features = np.asarray(features, dtype=np.float32)
    tgt = np.asarray(targets).astype(np.int64)

    feats = np.concatenate(
        [features[:, 0, :], features[:, 1, :], centers1], axis=0)   # [J, D]
    ftr = _truncate_fp22(feats)

    featsT_pad = np.zeros((D, JPAD), dtype=np.float32)
    featsT_pad[:, :J] = ftr.T

    tgt_all = np.concatenate([tgt, tgt, np.arange(C, dtype=np.int64)])
    onehot = np.zeros((JPAD, C), dtype=np.float32)
    onehot[np.arange(J), tgt_all] = 1.0
    # swizzle to SBUF layout: [p, c*C + k] = onehot[c*128 + p, k]
    oh_sw = np.ascontiguousarray(
        onehot.reshape(NCHUNK, 128, C).transpose(1, 0, 2).reshape(128, NCHUNK * C))

    E1 = _device_e1(featsT_pad, oh_sw).astype(np.float64)           # [C, 2B]

    # ---- host finalization (float64, O(B*D + C*B)) ----
    cnt = (2 * np.bincount(tgt, minlength=C) + 1).astype(np.float64)  # [C]
    u = 1.0 / cnt
    v = np.where(cnt > 1.0, 1.0 / np.maximum(cnt - 1.0, 1.0) - 1.0 / cnt, 0.0)
    t2b = tgt_all[:TWOB]
    M = cnt[t2b] - 1.0                                              # [2B]

    ftr64 = ftr.astype(np.float64)
    Sii = (ftr64[:TWOB] ** 2).sum(axis=1)                           # [2B]
    # Model of the device's diagonal term: ACT exp in f32, then the value the
    # reduce-matmul consumed (f32r path truncates the mantissa to 13 bits).
    eii_f32 = np.exp(INVT * Sii).astype(np.float32)
    if EII_MODE == "trunc":
        eii = _truncate_fp22(eii_f32).astype(np.float64)
    elif EII_MODE == "round":
        u32 = eii_f32.copy().view(np.uint32)
        u32 += np.uint32(0x1FF) + ((u32 >> np.uint32(10)) & np.uint32(1))
        u32 &= np.uint32(0xFFFFFC00)
        eii = u32.view(np.float32).astype(np.float64)
    else:
        eii = eii_f32.astype(np.float64)

    idx = np.arange(TWOB)
    A = u @ E1 + v[t2b] * E1[t2b, idx] - eii / M

    G = np.zeros((C, D), dtype=np.float64)
    np.add.at(G, tgt_all, ftr64)
    H = (ftr64[:TWOB] * G[t2b]).sum(axis=1) - Sii                   # [2B]

    loss_i = np.log(A) - INVT * H / M
    return np.asarray(loss_i.mean(), dtype=np.float32)


# revision 13
# speedup vs baseline: 1.1254x; 1.1254x over previous
"""BalSCL (balanced supervised contrastive loss) for Trainium2, 8 NeuronCores.

Math restructure (avoids all per-element mask work on device):
  tgt  = [targets, targets, arange(C)]            (length J = 2B + C = 8292)
  feats = [view0, view1, centers]                 [J, D], L2-normalized rows
  S[j, i] = feats_j . feats_i  (i over the 2B anchors), l = S / t  (offset-0
  convention: the reference's row-max subtraction cancels identically in
  loss_i = log(sum_j w_ij e^{l_ij}) - (sum_j mask_ij l_ij) / M_i).

  Reference per-row sum:  A_i = sum_{j != i} e^{l_ij} / (cnt[tgt_j] - m_ij)
  with m_ij = [tgt_j == tgt_i].  Since the weight only depends on (class of j,
  whether it equals class of i), A_i is a function of per-class exp-sums:
      E1[k, i] = sum_{j: tgt_j = k} e^{l_ij}
      A_i = sum_k E1[k,i]/cnt[k] + (1/(cnt[t_i]-1) - 1/cnt[t_i]) * E1[t_i, i]
            - e^{l_ii} / (cnt[t_i] - 1)
  The device computes ONLY E1 (matmul -> exp -> one-hot matmul); everything
  else (tiny O(B + C*B) work) happens on host in float64.

  The mask*logits sum is exact on host: sum_j mask_ij S_ij = f_i . G[t_i] - S_ii
  with G = per-class feature sums.

Device inputs are pre-truncated to FP22 (the PE's float32r multiply precision)
so the host knows the matmul inputs bit-exactly and can reproduce the e^{l_ii}
diagonal term it must subtract.

Sharding: anchors (the 2B = 8192 logit rows; free axis i on device) are split
1024 per core; feats^T / one-hot are replicated; host sums the scalar.
"""

import numpy as np

C = 100
B = 4096
D = 128
TWOB = 2 * B
J = TWOB + C            # 8292
NCHUNK = 65
JPAD = NCHUNK * 128     # 8320
NCORES = 8
PER = TWOB // NCORES    # 1024 anchors per core
INVT = 10.0             # 1 / temperature
EII_MODE = "trunc"      # how the device rounded the diagonal exp (calibrated)

_NC_CACHE = {}


def _truncate_fp22(a: np.ndarray) -> np.ndarray:
    """Truncate f32 mantissa to 13 bits (FP22) — what the PE reads for f32r."""
    b = np.ascontiguousarray(a, dtype=np.float32).copy()
    b.view(np.uint32)[...] &= np.uint32(0xFFFFFC00)
    return b


def _build_nc():
    import concourse.bacc as bacc
    import concourse.mybir as mybir
    import concourse.tile as tile

    f32 = mybir.dt.float32
    f32r = mybir.dt.float32r
    Exp = mybir.ActivationFunctionType.Exp

    nc = bacc.Bacc("TRN2", target_bir_lowering=False, debug=False,
                   num_devices=NCORES)

    ft_d = nc.dram_tensor("featsT", [D, JPAD], f32r, kind="ExternalInput")
    # one-hot pre-swizzled on host to SBUF layout [p, c*C + k] = onehot[128c+p, k]
    oh_d = nc.dram_tensor("onehot", [128, NCHUNK * C], f32r, kind="ExternalInput")
    an_d = nc.dram_tensor("anch", [D, PER], f32r, kind="ExternalInput")
    e1_d = nc.dram_tensor("e1", [C, PER], f32, kind="ExternalOutput")

    with tile.TileContext(nc) as tc:
        with (
            tc.tile_pool(name="big", bufs=1) as big,
            tc.tile_pool(name="epool", bufs=3) as epool,
            tc.tile_pool(name="outp", bufs=1) as outp,
            tc.tile_pool(name="spool", bufs=2, space="PSUM") as spool,
            tc.tile_pool(name="accpool", bufs=1, space="PSUM") as accpool,
        ):
            # progressive pieces, interleaved by deadline, all on the SP ring
            an = big.tile([D, PER], f32r, name="an")
            ft = big.tile([D, JPAD], f32r, name="ft")
            oh = big.tile([128, NCHUNK * C], f32r, name="oh")
            bounds = [0, 1, 3, 7, 15, 27, 45, NCHUNK]       # chunk indices
            nc.sync.dma_start(out=an[:, 0:512], in_=an_d[:, 0:512])
            for s in range(len(bounds) - 1):
                fa, fb = bounds[s] * 128, bounds[s + 1] * 128
                oa, ob = bounds[s] * C, bounds[s + 1] * C
                nc.sync.dma_start(out=ft[:, fa:fb], in_=ft_d[:, fa:fb])
                nc.sync.dma_start(out=oh[:, oa:ob], in_=oh_d[:, oa:ob])
                if s == 0:
                    nc.sync.dma_start(out=an[:, 512:1024],
                                      in_=an_d[:, 512:1024])

            E1s = [accpool.tile([C, 512], f32, name=f"E1_{blk}", tag=f"E1_{blk}")
                   for blk in range(2)]
            out_sb = outp.tile([C, PER], f32, name="out_sb")

            # job = (blk, chunks): 3 j-chunks per PSUM tile (3 banks), with
            # 1- and 2-chunk warmup groups so the pipeline fills fast
            def block_groups(warmup):
                gs = ([[0], [1, 2]] if warmup else [[0, 1, 2]])
                gs += [list(range(g, min(g + 3, NCHUNK)))
                       for g in range(3, NCHUNK, 3)]
                return gs

            jobs = [(0, g) for g in block_groups(True)] + \
                   [(1, g) for g in block_groups(False)]

            def emit_reduce(blk, chunks, e):
                for idx, c in enumerate(chunks):
                    nc.tensor.matmul(E1s[blk], lhsT=oh[:, c * C:(c + 1) * C],
                                     rhs=e[:, idx * 512:(idx + 1) * 512],
                                     start=(c == 0), stop=(c == NCHUNK - 1),
                                     skip_group_check=True)

            def emit_output(blk):
                half = out_sb[:, blk * 512:(blk + 1) * 512]
                nc.vector.tensor_copy(out=half, in_=E1s[blk])
                nc.sync.dma_start(out=e1_d[:, blk * 512:(blk + 1) * 512],
                                  in_=half)

            prev = None
            for j, (blk, chunks) in enumerate(jobs):
                w = len(chunks) * 512
                anh = an[:, blk * 512:(blk + 1) * 512]
                S = spool.tile([128, 1536], f32, name="S")
                for idx, c in enumerate(chunks):
                    nc.tensor.matmul(
                        S[:, idx * 512:(idx + 1) * 512],
                        lhsT=ft[:, c * 128:(c + 1) * 128], rhs=anh,
                        start=True, stop=True)
                e = epool.tile([128, 1536], f32r, name="e")
                if j == len(jobs) - 1:
                    # split the final exp so the last reduces start sooner
                    for idx in range(len(chunks)):
                        nc.scalar.activation(
                            out=e[:, idx * 512:(idx + 1) * 512],
                            in_=S[:, idx * 512:(idx + 1) * 512],
                            func=Exp, bias=0.0, scale=INVT)
                else:
                    nc.scalar.activation(out=e[:, 0:w], in_=S[:, 0:w],
                                         func=Exp, bias=0.0, scale=INVT)
                # pipeline skew: reduces for the previous tile come after this
                # tile's main matmuls so the PE never waits on the exp
                if prev is not None:
                    emit_reduce(*prev)
                    if prev[1][-1] == NCHUNK - 1:     # previous block finished
                        emit_output(prev[0])
                prev = (blk, chunks, e)
            emit_reduce(*prev)
            emit_output(prev[0])

    nc.compile()
    return nc


def get_nc():
    if "nc" not in _NC_CACHE:
        _NC_CACHE["nc"] = _build_nc()
    return _NC_CACHE["nc"]


def _device_e1(featsT_pad: np.ndarray, oh_sw: np.ndarray) -> np.ndarray:
    """Run the SPMD kernel on 8 cores; return E1 [C, 2B] float32."""
    from concourse.bass_utils import run_bass_kernel_spmd

    nc = get_nc()
    in_maps = []
    for core in range(NCORES):
        anch = np.ascontiguousarray(featsT_pad[:, core * PER:(core + 1) * PER])
        in_maps.append({"featsT": featsT_pad, "onehot": oh_sw, "anch": anch})
    res = run_bass_kernel_spmd(nc, in_maps, core_ids=list(range(NCORES)))
    return np.concatenate([res.results[c]["e1"] for c in range(NCORES)], axis=1)


def kernel(centers1: np.ndarray, features: np.ndarray,
           targets: np.ndarray) -> np.ndarray:
    centers1 = np.asarray(centers1, dtype=np.float32)
    features = np.asarray(features, dtype=np.float32)
    tgt = np.asarray(targets).astype(np.int64)

    feats = np.concatenate(
        [features[:, 0, :], features[:, 1, :], centers1], axis=0)   # [J, D]
    ftr = _truncate_fp22(feats)

    featsT_pad = np.zeros((D, JPAD), dtype=np.float32)
    featsT_pad[:, :J] = ftr.T

    tgt_all = np.concatenate([tgt, tgt, np.arange(C, dtype=np.int64)])
    onehot = np.zeros((JPAD, C), dtype=np.float32)
    onehot[np.arange(J), tgt_all] = 1.0
    # swizzle to SBUF layout: [p, c*C + k] = onehot[c*128 + p, k]
    oh_sw = np.ascontiguousarray(
        onehot.reshape(NCHUNK, 128, C).transpose(1, 0, 2).reshape(128, NCHUNK * C))

    E1 = _device_e1(featsT_pad, oh_sw).astype(np.float64)           # [C, 2B]

    # ---- host finalization (float64, O(B*D + C*B)) ----
    cnt = (2 * np.bincount(tgt, minlength=C) + 1).astype(np.float64)  # [C]
    u = 1.0 / cnt
    v = np.where(cnt > 1.0, 1.0 / np.maximum(cnt - 1.0, 1.0) - 1.0 / cnt, 0.0)
    t2b = tgt_all[:TWOB]
    M = cnt[t2b] - 1.0                                              # [2B]

    ftr64 = ftr.astype(np.float64)
    Sii = (ftr64[:TWOB] ** 2).sum(axis=1)                           # [2B]
    # Model of the device's diagonal term: ACT exp in f32, then the value the
    # reduce-matmul consumed (f32r path truncates the mantissa to 13 bits).
    eii_f32 = np.exp(INVT * Sii).astype(np.float32)
    if EII_MODE == "trunc":
        eii = _truncate_fp22(eii_f32).astype(np.float64)
    elif EII_MODE == "round":
        u32 = eii_f32.copy().view(np.uint32)
        u32 += np.uint32(0x1FF) + ((u32 >> np.uint32(10)) & np.uint32(1))
        u32 &= np.uint32(0xFFFFFC00)
        eii = u32.view(np.float32).astype(np.float64)
    else:
        eii = eii_f32.astype(np.float64)

    idx = np.arange(TWOB)
    A = u @ E1 + v[t2b] * E1[t2b, idx] - eii / M

    G = np.zeros((C, D), dtype=np.float64)
    np.add.at(G, tgt_all, ftr64)
    H = (ftr64[:TWOB] * G[t2b]).sum(axis=1) - Sii                   # [2B]

    loss_i = np.log(A) - INVT * H / M
    return np.asarray(loss_i.mean(), dtype=np.float32)


# revision 32
# speedup vs baseline: 1.1639x; 1.0342x over previous
"""BalSCL (balanced supervised contrastive loss) for Trainium2, 8 NeuronCores.

Math restructure (avoids all per-element mask work on device):
  tgt  = [targets, targets, arange(C)]            (length J = 2B + C = 8292)
  feats = [view0, view1, centers]                 [J, D], L2-normalized rows
  S[j, i] = feats_j . feats_i  (i over the 2B anchors), l = S / t  (offset-0
  convention: the reference's row-max subtraction cancels identically in
  loss_i = log(sum_j w_ij e^{l_ij}) - (sum_j mask_ij l_ij) / M_i).

  Reference per-row sum:  A_i = sum_{j != i} e^{l_ij} / (cnt[tgt_j] - m_ij)
  with m_ij = [tgt_j == tgt_i].  Since the weight only depends on (class of j,
  whether it equals class of i), A_i is a function of per-class exp-sums:
      E1[k, i] = sum_{j: tgt_j = k} e^{l_ij}
      A_i = sum_k E1[k,i]/cnt[k] + (1/(cnt[t_i]-1) - 1/cnt[t_i]) * E1[t_i, i]
            - e^{l_ii} / (cnt[t_i] - 1)
  The device computes ONLY E1 (matmul -> exp -> one-hot matmul); everything
  else (tiny O(B + C*B) work) happens on host in float64.

  The mask*logits sum is exact on host: sum_j mask_ij S_ij = f_i . G[t_i] - S_ii
  with G = per-class feature sums.

Device inputs are pre-rounded to bf16 on the host (the PE upconverts bf16 to
FP22 exactly), so the host knows the matmul inputs bit-exactly and can
reproduce the e^{l_ii} diagonal term it must subtract from the device's
class sums (the exp output is likewise rounded to bf16 by the ScalarE write,
which the host replicates when forming e^{l_ii}).

Sharding: anchors (the 2B = 8192 logit rows; free axis i on device) are split
1024 per core; feats^T / one-hot are replicated; host sums the scalar.
"""

import numpy as np

C = 100
B = 4096
D = 128
TWOB = 2 * B
J = TWOB + C            # 8292
NCHUNK = 65
JPAD = NCHUNK * 128     # 8320
NCORES = 8
PER = TWOB // NCORES    # 1024 anchors per core
INVT = 10.0             # 1 / temperature
EII_MODE = "bf16"       # how the device rounded the diagonal exp (calibrated)

_NC_CACHE = {}


def _bf16(a: np.ndarray):
    import ml_dtypes
    return np.asarray(a, dtype=np.float32).astype(ml_dtypes.bfloat16)


def _build_nc():
    import concourse.bacc as bacc
    import concourse.mybir as mybir
    import concourse.tile as tile

    f32 = mybir.dt.float32
    bf16 = mybir.dt.bfloat16
    Exp = mybir.ActivationFunctionType.Exp

    nc = bacc.Bacc("TRN2", target_bir_lowering=False, debug=False,
                   num_devices=NCORES)

    ft_d = nc.dram_tensor("featsT", [D, JPAD], bf16, kind="ExternalInput")
    # one-hot pre-swizzled on host to SBUF layout [p, c*C + k] = onehot[128c+p, k]
    oh_d = nc.dram_tensor("onehot", [128, NCHUNK * C], bf16, kind="ExternalInput")
    an_d = nc.dram_tensor("anch", [D, PER], bf16, kind="ExternalInput")
    e1_d = nc.dram_tensor("e1", [C, PER], f32, kind="ExternalOutput")

    with tile.TileContext(nc) as tc:
        with (
            tc.tile_pool(name="big", bufs=1) as big,
            tc.tile_pool(name="epool", bufs=4) as epool,
            tc.tile_pool(name="outp", bufs=1) as outp,
            tc.tile_pool(name="spool", bufs=2, space="PSUM") as spool,
            tc.tile_pool(name="accpool", bufs=1, space="PSUM") as accpool,
        ):
            # PE warmup: dummy matmuls on a zeroed scratch tile while the
            # input DMAs stream in, so the HAM clock-gate opens before the
            # first real matmul (and the cost model's p-state ramp likewise)
            warm = big.tile([128, 256], bf16, name="warm")
            nc.gpsimd.memset(warm, 0.0)

            # progressive pieces, interleaved by deadline, all on the SP ring
            an = big.tile([D, PER], bf16, name="an")
            ft = big.tile([D, JPAD], bf16, name="ft")
            oh = big.tile([128, NCHUNK * C], bf16, name="oh")
            bounds = [0, 5, 15, 35, NCHUNK]       # chunk indices
            nc.sync.dma_start(out=an[:, 0:512], in_=an_d[:, 0:512])
            for s in range(len(bounds) - 1):
                fa, fb = bounds[s] * 128, bounds[s + 1] * 128
                oa, ob = bounds[s] * C, bounds[s + 1] * C
                nc.sync.dma_start(out=ft[:, fa:fb], in_=ft_d[:, fa:fb])
                nc.sync.dma_start(out=oh[:, oa:ob], in_=oh_d[:, oa:ob])
                if s == 0:
                    nc.sync.dma_start(out=an[:, 512:1024],
                                      in_=an_d[:, 512:1024])

            E1s = [accpool.tile([C, 512], f32, name=f"E1_{blk}", tag=f"E1_{blk}")
                   for blk in range(2)]
            out_sb = outp.tile([C, PER], f32, name="out_sb")

            warm_S = spool.tile([128, 1536], f32, name="S")
            for _ in range(8):
                nc.tensor.matmul(warm_S[:, 0:256], lhsT=warm[:, 0:128],
                                 rhs=warm, start=True, stop=True,
                                 skip_group_check=True)

            # job = (blk, chunks): 3 j-chunks per PSUM tile (3 banks), with
            # 1- and 2-chunk warmup groups so the pipeline fills fast
            def block_groups(warmup):
                gs = ([[0], [1, 2]] if warmup else [[0, 1, 2]])
                gs += [list(range(g, min(g + 3, NCHUNK)))
                       for g in range(3, NCHUNK, 3)]
                return gs

            jobs = [(0, g) for g in block_groups(True)] + \
                   [(1, g) for g in block_groups(False)]

            def emit_reduce(blk, chunks, e):
                for idx, c in enumerate(chunks):
                    nc.tensor.matmul(E1s[blk], lhsT=oh[:, c * C:(c + 1) * C],
                                     rhs=e[:, idx * 512:(idx + 1) * 512],
                                     start=(c == 0), stop=(c == NCHUNK - 1),
                                     skip_group_check=True)

            def emit_output(blk):
                half = out_sb[:, blk * 512:(blk + 1) * 512]
                nc.vector.tensor_copy(out=half, in_=E1s[blk])
                nc.sync.dma_start(out=e1_d[:, blk * 512:(blk + 1) * 512],
                                  in_=half)

            prev = None
            for j, (blk, chunks) in enumerate(jobs):
                w = len(chunks) * 512
                anh = an[:, blk * 512:(blk + 1) * 512]
                S = spool.tile([128, 1536], f32, name="S")
                for idx, c in enumerate(chunks):
                    nc.tensor.matmul(
                        S[:, idx * 512:(idx + 1) * 512],
                        lhsT=ft[:, c * 128:(c + 1) * 128], rhs=anh,
                        start=True, stop=True)
                e = epool.tile([128, 1536], bf16, name="e")
                if j == len(jobs) - 1:
                    # split the final exp and interleave its reduces so the
                    # last reduce starts as soon as possible
                    if prev is not None:
                        emit_reduce(*prev)
                        if prev[1][-1] == NCHUNK - 1:
                            emit_output(prev[0])
                    for idx, c in enumerate(chunks):
                        nc.scalar.activation(
                            out=e[:, idx * 512:(idx + 1) * 512],
                            in_=S[:, idx * 512:(idx + 1) * 512],
                            func=Exp, bias=0.0, scale=INVT)
                        nc.tensor.matmul(E1s[blk],
                                         lhsT=oh[:, c * C:(c + 1) * C],
                                         rhs=e[:, idx * 512:(idx + 1) * 512],
                                         start=(c == 0),
                                         stop=(c == NCHUNK - 1),
                                         skip_group_check=True)
                    emit_output(blk)
                    prev = None
                    continue
                nc.scalar.activation(out=e[:, 0:w], in_=S[:, 0:w],
                                     func=Exp, bias=0.0, scale=INVT)
                # pipeline skew: reduces for the previous tile come after this
                # tile's main matmuls so the PE never waits on the exp
                if prev is not None:
                    emit_reduce(*prev)
                    if prev[1][-1] == NCHUNK - 1:     # previous block finished
                        emit_output(prev[0])
                prev = (blk, chunks, e)
            if prev is not None:
                emit_reduce(*prev)
                emit_output(prev[0])

    nc.compile()
    return nc


def get_nc():
    if "nc" not in _NC_CACHE:
        _NC_CACHE["nc"] = _build_nc()
    return _NC_CACHE["nc"]


def _make_in_maps(featsT_pad, oh_sw):
    in_maps = []
    for core in range(NCORES):
        anch = np.ascontiguousarray(featsT_pad[:, core * PER:(core + 1) * PER])
        in_maps.append({"featsT": featsT_pad, "onehot": oh_sw, "anch": anch})
    return in_maps


def _cached_pjrt_runner():
    """Build (once) a jitted shard_map executor mirroring
    concourse.bass2jax.run_bass_via_pjrt, so repeated kernel() calls reuse
    the compiled executable instead of re-tracing per call."""
    import jax
    import numpy as _np
    from jax.sharding import Mesh, PartitionSpec
    from jax.experimental.shard_map import shard_map
    import concourse.mybir as mybir
    from concourse import bass2jax as b2j

    nc = get_nc()
    b2j.install_neuronx_cc_hook()
    partition_name = (nc.partition_id_tensor.name
                      if nc.partition_id_tensor else None)
    in_names, out_names, out_avals, zero_outs = [], [], [], []
    for alloc in nc.m.functions[0].allocations:
        if not isinstance(alloc, mybir.MemoryLocationSet):
            continue
        name = alloc.memorylocations[0].name
        if alloc.kind == "ExternalInput":
            if name != partition_name:
                in_names.append(name)
        elif alloc.kind == "ExternalOutput":
            shape = tuple(alloc.tensor_shape)
            dtype = mybir.dt.np(alloc.dtype)
            out_names.append(name)
            out_avals.append(jax.core.ShapedArray(shape, dtype))
            zero_outs.append(_np.zeros(shape, dtype))
    n_params = len(in_names)
    all_names = list(in_names) + list(out_names)
    if partition_name is not None:
        all_names.append(partition_name)
    donate = tuple(range(n_params, n_params + len(out_names)))

    def _body(*args):
        operands = list(args)
        if partition_name is not None:
            operands.append(b2j.partition_id_tensor())
        outs = b2j._bass_exec_p.bind(
            *operands,
            out_avals=tuple(out_avals),
            in_names=tuple(all_names),
            out_names=tuple(out_names),
            lowering_input_output_aliases=(),
            sim_require_finite=True,
            sim_require_nnan=True,
            nc=nc,
        )
        return tuple(outs)

    devices = jax.devices()[:NCORES]
    mesh = Mesh(_np.asarray(devices), ("core",))
    in_specs = (PartitionSpec("core"),) * (n_params + len(out_names))
    out_specs = (PartitionSpec("core"),) * len(out_names)
    sharded = jax.jit(
        shard_map(_body, mesh=mesh, in_specs=in_specs, out_specs=out_specs,
                  check_rep=False),
        donate_argnums=donate, keep_unused=True)

    def run(in_maps):
        per_core = [[_np.asarray(m[nm]) for nm in in_names] for m in in_maps]
        concat_in = [
            _np.concatenate([per_core[c][i] for c in range(NCORES)], axis=0)
            for i in range(n_params)
        ]
        concat_zeros = [
            _np.zeros((NCORES * z.shape[0], *z.shape[1:]), z.dtype)
            for z in zero_outs
        ]
        out_arrs = sharded(*concat_in, *concat_zeros)
        return [
            {nm: _np.asarray(out_arrs[i]).reshape(NCORES, *out_avals[i].shape)[c]
             for i, nm in enumerate(out_names)}
            for c in range(NCORES)
        ]

    return run


def _device_e1(featsT_pad: np.ndarray, oh_sw: np.ndarray) -> np.ndarray:
    """Run the SPMD kernel on 8 cores; return E1 [C, 2B] float32."""
    in_maps = _make_in_maps(featsT_pad, oh_sw)
    try:
        if "runner" not in _NC_CACHE:
            _NC_CACHE["runner"] = _cached_pjrt_runner()
        results = _NC_CACHE["runner"](in_maps)
    except Exception:
        _NC_CACHE.pop("runner", None)
        from concourse.bass_utils import run_bass_kernel_spmd
        results = run_bass_kernel_spmd(
            get_nc(), in_maps, core_ids=list(range(NCORES))).results
    return np.concatenate([results[c]["e1"] for c in range(NCORES)], axis=1)


def kernel(centers1: np.ndarray, features: np.ndarray,
           targets: np.ndarray) -> np.ndarray:
    centers1 = np.asarray(centers1, dtype=np.float32)
    features = np.asarray(features, dtype=np.float32)
    tgt = np.asarray(targets).astype(np.int64)

    import ml_dtypes
    feats = np.concatenate(
        [features[:, 0, :], features[:, 1, :], centers1], axis=0)   # [J, D]
    ftr_b = _bf16(feats)                      # what the device multiplies
    ftr = ftr_b.astype(np.float32)

    featsT_pad = np.zeros((D, JPAD), dtype=ml_dtypes.bfloat16)
    featsT_pad[:, :J] = ftr_b.T

    tgt_all = np.concatenate([tgt, tgt, np.arange(C, dtype=np.int64)])
    onehot = np.zeros((JPAD, C), dtype=ml_dtypes.bfloat16)
    onehot[np.arange(J), tgt_all] = 1.0
    # swizzle to SBUF layout: [p, c*C + k] = onehot[c*128 + p, k]
    oh_sw = np.ascontiguousarray(
        onehot.reshape(NCHUNK, 128, C).transpose(1, 0, 2).reshape(128, NCHUNK * C))

    E1 = _device_e1(featsT_pad, oh_sw).astype(np.float64)           # [C, 2B]

    # ---- host finalization (float64, O(B*D + C*B)) ----
    cnt = (2 * np.bincount(tgt, minlength=C) + 1).astype(np.float64)  # [C]
    u = 1.0 / cnt
    v = np.where(cnt > 1.0, 1.0 / np.maximum(cnt - 1.0, 1.0) - 1.0 / cnt, 0.0)
    t2b = tgt_all[:TWOB]
    M = cnt[t2b] - 1.0                                              # [2B]

    ftr64 = ftr.astype(np.float64)
    Sii = (ftr64[:TWOB] ** 2).sum(axis=1)                           # [2B]
    # Model of the device's diagonal term: ACT exp in f32, rounded to bf16 on
    # write (the reduce matmul consumed the bf16 value).
    eii_f32 = np.exp(INVT * Sii).astype(np.float32)
    if EII_MODE == "bf16":
        eii = _bf16(eii_f32).astype(np.float64)
    else:
        eii = eii_f32.astype(np.float64)

    idx = np.arange(TWOB)
    A = u @ E1 + v[t2b] * E1[t2b, idx] - eii / M

    G = np.zeros((C, D), dtype=np.float64)
    np.add.at(G, tgt_all, ftr64)
    H = (ftr64[:TWOB] * G[t2b]).sum(axis=1) - Sii                   # [2B]

    loss_i = np.log(A) - INVT * H / M
    return np.asarray(loss_i.mean(), dtype=np.float32)
